# revision 1
# baseline (speedup 1.0000x reference)
"""MDTA Trainium2 kernel: 8 cores = 4 samples x 2 head-groups.

v1: unfolded qkv (1x1 + banded depthwise-3x3 fused in SBUF), norms fused
into producer stages, gram via DMA-transpose + SBUF-resident q^T/k^T,
bf16 inputs/outputs, per-iteration memsets eliminated, PSUM-evacuation
spread across Activation/DVE/Pool engines.
"""
import numpy as np
import ml_dtypes
import json as _json
import concourse.bass as bass

# Patch Bass.to_json_bytes: split multi-sem-waits onto same-engine NoOps
# (this walrus build rejects instructions with >1 sync wait).
_orig_tjb = bass.Bass.to_json_bytes
_wctr = [0]

def _split_waits(block):
    out = []
    for ins in block.get("instructions", []):
        si = ins.get("sync_info")
        waits = (si or {}).get("on_wait") or []
        if len(waits) > 1:
            si["on_wait"] = waits[-1:]
            for w in waits[:-1]:
                _wctr[0] += 1
                out.append({"debug": ins.get("debug", 0), "engine": ins["engine"],
                            "ins": [], "outs": [], "name": f"wsplit-{_wctr[0]}",
                            "opcode": "NoOp",
                            "sync_info": {"on_update": [], "on_wait": [w]}})
        out.append(ins)
    block["instructions"] = out
    for sub in block.get("blocks", []):
        _split_waits(sub)

def _patched_tjb(self):
    d = _json.loads(_orig_tjb(self))
    for fn in d.get("functions", []):
        for b in fn.get("blocks", []):
            _split_waits(b)
    return _json.dumps(d).encode()

if not getattr(bass.Bass, "_waitfix_done", False):
    bass.Bass.to_json_bytes = _patched_tjb
    bass.Bass._waitfix_done = True
import concourse.mybir as mybir
from concourse.tile import TileContext
from concourse.bass_utils import run_bass_kernel_spmd

BF = mybir.dt.bfloat16
F32 = mybir.dt.float32
H, W, C = 192, 192, 256
HW = H * W
S = 98  # subband size

DEC_LO = np.array([0.035226291882100656, -0.085441273882241486, -0.13501102001039084,
                   0.45987750211933132, 0.80689150931333875, 0.33267055295095688], dtype=np.float64)
DEC_HI = np.array([-0.33267055295095688, 0.80689150931333875, -0.45987750211933132,
                   -0.13501102001039084, 0.085441273882241486, 0.035226291882100656], dtype=np.float64)
H0A = DEC_LO[::-1].copy()
H1A = DEC_HI[::-1].copy()
G0S = DEC_LO.copy()  # REC_LO reversed = DEC_LO
G1S = np.array([0.035226291882100656, 0.085441273882241486, -0.13501102001039084,
                -0.45987750211933132, 0.80689150931333875, -0.33267055295095688], dtype=np.float64)[::-1].copy()


STAGE_MARKS = []


def _mark(nc, stage):
    STAGE_MARKS.append((int(nc.get_next_instruction_name().split("-")[1]), stage))


_rot = [0]


def _copy(nc, out, in_):
    # PSUM evacuation: GPSIMD/Pool cannot access PSUM, so rotate Act/DVE only.
    r = _rot[0] % 2
    _rot[0] += 1
    if r == 0:
        nc.scalar.copy(out, in_)
    else:
        nc.vector.tensor_copy(out, in_)


def build_core_kernel():
    nc = bass.Bass("TRN2")
    STAGE_MARKS.clear()
    _rot[0] = 0
    # inputs (per core)
    xk0 = nc.dram_tensor("xk0", [128, H, W], BF, kind="ExternalInput")
    xk1 = nc.dram_tensor("xk1", [128, H, W], BF, kind="ExternalInput")
    xq = nc.dram_tensor("xq", [128, H, W], BF, kind="ExternalInput")  # local 128 ch
    w1x1 = nc.dram_tensor("w1x1", [2, 128, 256], BF, kind="ExternalInput")  # [in-half, in, out(k|v)]
    dwk = nc.dram_tensor("dwk", [9, 128, 128], BF, kind="ExternalInput")  # diag dw taps, k half
    dwv = nc.dram_tensor("dwv", [9, 128, 128], BF, kind="ExternalInput")  # diag dw taps, v half
    taps_ab = nc.dram_tensor("taps_ab", [12, 128, 128], BF, kind="ExternalInput")
    taps_de = nc.dram_tensor("taps_de", [12, 128, 128], BF, kind="ExternalInput")
    dwq = nc.dram_tensor("dwq", [4, 9, 128, 128], BF, kind="ExternalInput")
    projlt = nc.dram_tensor("projlt", [128, 256], BF, kind="ExternalInput")
    tempv = nc.dram_tensor("tempv", [128, 1], F32, kind="ExternalInput")
    identb = nc.dram_tensor("identb", [128, 128], BF, kind="ExternalInput")
    y = nc.dram_tensor("y", [2, 128, HW], BF, kind="ExternalOutput")
    # DRAM scratch
    vd = nc.dram_tensor("vd", [128, HW], BF)
    qd = nc.dram_tensor("qd", [128, H, W], BF)
    loh = nc.dram_tensor("loh", [128, 2, H, S], BF)
    subb = nc.dram_tensor("subb", [128, 4, S, 100], BF)  # W-padded (cols 0,99 zeroed once)
    zq = nc.dram_tensor("zq", [128, 4, S, S], BF)

    with TileContext(nc) as tc:
        with tc.tile_pool(name="const", bufs=1) as cpool:
            # ---- constants
            t_ab = cpool.tile([128, 12, 128], BF)
            nc.scalar.dma_start(out=t_ab[:, :, :], in_=taps_ab.rearrange("t p c -> p t c"))
            t_de = cpool.tile([128, 12, 128], BF)
            nc.scalar.dma_start(out=t_de[:, :, :], in_=taps_de.rearrange("t p c -> p t c"))
            t_dw = cpool.tile([128, 36, 128], BF)
            nc.scalar.dma_start(out=t_dw[:, :, :], in_=dwq.rearrange("s t p c -> p (s t) c"))
            t_w1 = cpool.tile([128, 2, 256], BF)
            nc.sync.dma_start(out=t_w1[:, :, :], in_=w1x1.rearrange("h p c -> p h c"))
            t_dwk = cpool.tile([128, 9, 128], BF)
            nc.sync.dma_start(out=t_dwk[:, :, :], in_=dwk.rearrange("t p c -> p t c"))
            t_dwv = cpool.tile([128, 9, 128], BF)
            nc.sync.dma_start(out=t_dwv[:, :, :], in_=dwv.rearrange("t p c -> p t c"))
            t_proj = cpool.tile([128, 256], BF)
            nc.scalar.dma_start(out=t_proj[:, :], in_=projlt[:, :])
            t_id = cpool.tile([128, 128], BF)
            nc.scalar.dma_start(out=t_id[:, :], in_=identb[:, :])
            t_temp = cpool.tile([128, 1], F32)
            nc.sync.dma_start(out=t_temp[:, :], in_=tempv[:, :])

            knp = cpool.tile([128, 16], F32)
            qnp = cpool.tile([128, 20], F32)
            nc.vector.memset(knp[:, :], 0)
            nc.vector.memset(qnp[:, :], 0)
            mt_ = cpool.tile([128, 256], BF)     # attention+proj matrix (gram phase -> y phase)
            kdT = cpool.tile([128, 288, 128], BF)  # transposed k, SBUF-resident

            _mark(nc, "kv")
            # ======== kv: 1x1 (C=256 -> k|v 128+128) + depthwise 3x3, fused per 12-row band
            BKV, NB = 12, 16
            with tc.tile_pool(name="kvw", bufs=2) as kp, \
                 tc.tile_pool(name="kvps", bufs=1, space="PSUM") as pp1:
                for b in range(NB):
                    r0 = BKV * b
                    xb0 = kp.tile([128, 14, 192], BF, tag="xb0")
                    xb1 = kp.tile([128, 14, 192], BF, tag="xb1")
                    v0, v1 = max(0, r0 - 1), min(H, r0 + BKV + 1)
                    if b == 0:
                        nc.vector.memset(xb0[:, 0, :], 0)
                        nc.vector.memset(xb1[:, 0, :], 0)
                    if b == NB - 1:
                        nc.vector.memset(xb0[:, 13, :], 0)
                        nc.vector.memset(xb1[:, 13, :], 0)
                    nc.sync.dma_start(out=xb0[:, v0 - (r0 - 1):v1 - (r0 - 1), :], in_=xk0[:, v0:v1, :])
                    nc.sync.dma_start(out=xb1[:, v0 - (r0 - 1):v1 - (r0 - 1), :], in_=xk1[:, v0:v1, :])
                    kvp = [kp.tile([128, 14, 194], BF, tag="kvp0", name="kvp0"),
                           kp.tile([128, 14, 194], BF, tag="kvp1", name="kvp1")]
                    for mt in range(2):  # zero W-pad columns (tiny, every band)
                        nc.vector.memset(kvp[mt][:, :, 0:1], 0)
                        nc.vector.memset(kvp[mt][:, :, 193:194], 0)
                    for mt in range(2):
                        for i in range(7):
                            ps = pp1.tile([128, 2, 192], F32, tag="ps1", bufs=2, name="ps1")
                            nc.tensor.matmul(ps[:, :, :], t_w1[:, 0, 128 * mt:128 * mt + 128],
                                             xb0[:, 2 * i:2 * i + 2, :], start=True, stop=False)
                            nc.tensor.matmul(ps[:, :, :], t_w1[:, 1, 128 * mt:128 * mt + 128],
                                             xb1[:, 2 * i:2 * i + 2, :], start=False, stop=True)
                            _copy(nc, kvp[mt][:, 2 * i:2 * i + 2, 1:193], ps[:, :, :])
                    for mt in range(2):
                        wt = t_dwk if mt == 0 else t_dwv
                        psd = [pp1.tile([128, 2, 192], F32, tag="psdw", bufs=6, name="psd")
                               for _ in range(6)]
                        for t9 in range(9):
                            u, v = divmod(t9, 3)
                            for j in range(6):
                                nc.tensor.matmul(psd[j][:, :, :], wt[:, t9, :],
                                                 kvp[mt][:, 2 * j + u:2 * j + u + 2, v:v + 192],
                                                 start=(t9 == 0), stop=(t9 == 8))
                        out = kp.tile([128, 12, 192], BF, tag=f"okv{mt}", name="okv")
                        for j in range(6):
                            _copy(nc, out[:, 2 * j:2 * j + 2, :], psd[j][:, :, :])
                        if mt == 0:
                            sqk = kp.tile([128, 12, 192], BF, tag="sqk")
                            nc.vector.scalar_tensor_tensor(sqk[:, :, :], out[:, :, :], 1.0,
                                                           out[:, :, :], mybir.AluOpType.mult,
                                                           mybir.AluOpType.mult,
                                                           accum_out=knp[:, b:b + 1])
                            nc.sync.dma_start_transpose(
                                out=kdT[:, 18 * b:18 * b + 18, :],
                                in_=out.rearrange("p r w -> p (r w)"))
                        else:
                            nc.sync.dma_start(out=vd[:, r0 * W:(r0 + BKV) * W],
                                              in_=out.rearrange("p r w -> p (r w)"))
                    if b == 0:  # zero subb (pad cols) via Act queue, overlapped with kv
                        zsrc = kp.tile([128, 1960], BF, tag="zsrc", bufs=1)
                        nc.vector.memset(zsrc[:, :], 0)
                        for sb in range(4):
                            for j in range(5):
                                nc.scalar.dma_start(
                                    out=subb[:, sb, :, :].rearrange("p r w -> p (r w)")[:, 1960 * j:1960 * j + 1960],
                                    in_=zsrc[:, :])

            # k-norm chain early (knp final after kv) - off the attention tail
            kn = cpool.tile([128, 1], F32)
            rk = cpool.tile([128, 1], F32)
            nc.vector.tensor_reduce(kn[:, :], knp[:, :], axis=mybir.AxisListType.X,
                                    op=mybir.AluOpType.add)
            nc.scalar.sqrt(kn[:, :], kn[:, :])
            nc.vector.tensor_scalar_max(kn[:, :], kn[:, :], 1e-12)
            nc.vector.reciprocal(rk[:, :], kn[:, :])

            # ======== wavelet query path (c-parts diag matmuls)
            with tc.tile_pool(name="ww", bufs=2) as pool, \
                 tc.tile_pool(name="wps", bufs=8, space="PSUM") as pp:
                _mark(nc, "stageA")
                # stage A: W-analysis (x -> lo/hi)
                RA = 5
                for it, r0 in enumerate(range(0, H, RA)):
                    rr = min(RA, H - r0)
                    xt = pool.tile([128, RA, 202], BF, tag="xa", bufs=4)
                    nc.vector.memset(xt[:, :rr, 0:4], 0)
                    nc.vector.memset(xt[:, :rr, 196:202], 0)
                    nc.sync.dma_start(out=xt[:, :rr, 4:196], in_=xq[:, r0:r0 + rr, :])
                    for f in range(2):
                        ps = pp.tile([128, RA, S], F32, tag="ps")
                        for t in range(6):
                            rhs = xt[:, :rr, t:t + 196].rearrange("p r (j two) -> p two r j", two=2)[:, 0]
                            nc.tensor.matmul(ps[:, :rr, :], t_ab[:, 6 * f + t, :], rhs,
                                             start=(t == 0), stop=(t == 5))
                        ot = pool.tile([128, RA, S], BF, tag="oA", bufs=3)
                        _copy(nc, ot[:, :rr, :], ps[:, :rr, :])
                        nc.sync.dma_start(out=loh[:, f, r0:r0 + rr, :], in_=ot[:, :rr, :])

                _mark(nc, "stageB")
                # stage B: H-analysis (lo/hi -> 4 subbands)
                RB = 5
                for m0 in range(0, S, RB):
                    mm = min(RB, S - m0)
                    lo_r0 = 2 * m0 - 4
                    nrows = 2 * mm + 5
                    bt = pool.tile([128, 2, 2 * RB + 5, S], BF, tag="xb", bufs=4)
                    v0 = max(0, lo_r0)
                    v1 = min(H, lo_r0 + nrows)
                    if v0 > lo_r0 or v1 < lo_r0 + nrows:
                        nc.vector.memset(bt[:, :, :, :], 0)
                    nc.sync.dma_start(out=bt[:, :, v0 - lo_r0:v1 - lo_r0, :],
                                      in_=loh[:, :, v0:v1, :])
                    for sb in range(4):
                        f_h, src = (sb % 2), (sb // 2)
                        ps = pp.tile([128, RB, S], F32, tag="ps")
                        for t in range(6):
                            rhs = bt[:, src, t:t + 2 * mm, :].rearrange("p (m two) j -> p two m j", two=2)[:, 0]
                            nc.tensor.matmul(ps[:, :mm, :], t_ab[:, 6 * f_h + t, :], rhs,
                                             start=(t == 0), stop=(t == 5))
                        ot = pool.tile([128, RB, S], BF, tag="oB", bufs=3)
                        _copy(nc, ot[:, :mm, :], ps[:, :mm, :])
                        nc.sync.dma_start(out=subb[:, sb, m0:m0 + mm, 1:99], in_=ot[:, :mm, :])

                _mark(nc, "stageC")
                # stage C: depthwise 3x3, all 4 subbands per row-chunk (one shared load)
                RC_ = 5
                for it, m0 in enumerate(range(0, S, RC_)):
                    mm = min(RC_, S - m0)
                    ct = pool.tile([128, 4, RC_ + 2, 100], BF, tag="xc", bufs=4)
                    v0 = max(0, m0 - 1)
                    v1 = min(S, m0 + mm + 1)
                    if v0 > m0 - 1 or v1 < m0 + mm + 1:
                        nc.vector.memset(ct[:, :, :, :], 0)
                    nc.sync.dma_start(out=ct[:, :, v0 - (m0 - 1):v1 - (m0 - 1), :],
                                      in_=subb[:, :, v0:v1, :])
                    for sb in range(4):
                        ps = pp.tile([128, RC_, S], F32, tag="ps")
                        for u in range(3):
                            for v in range(3):
                                t = 3 * u + v
                                nc.tensor.matmul(ps[:, :mm, :], t_dw[:, 9 * sb + t, :],
                                                 ct[:, sb, u:u + mm, v:v + 98],
                                                 start=(t == 0), stop=(t == 8))
                        ot = pool.tile([128, RC_, S], BF, tag="oC", bufs=3)
                        _copy(nc, ot[:, :mm, :], ps[:, :mm, :])
                        nc.sync.dma_start(out=zq[:, sb, m0:m0 + mm, :], in_=ot[:, :mm, :])

                _mark(nc, "stageD")
                # stages D+E fused: H-synthesis to an SBUF band, immediately W-synthesized
                # into q rows (no synth DRAM round-trip)
                RD = 5
                for a0 in range(0, 96, RD):
                    aa = min(RD, 96 - a0)
                    dt_ = pool.tile([128, 4, RD + 2, S], BF, tag="xd", bufs=5)
                    v1 = min(S, a0 + aa + 2)
                    nc.sync.dma_start(out=dt_[:, :, :v1 - a0, :],
                                      in_=zq[:, :, a0:v1, :])
                    sy = pool.tile([128, 2, 2, RD, S], BF, tag="sy", bufs=2)
                    for fo in range(2):
                        for pr in range(2):
                            ps = pp.tile([128, RD, S], F32, tag="ps")
                            for src in range(2):
                                for d in range(3):
                                    ti = 6 * src + (2 * d + 1 - pr)
                                    nc.tensor.matmul(ps[:, :aa, :], t_de[:, ti, :],
                                                     dt_[:, 2 * fo + src, d:d + aa, :],
                                                     start=(src == 0 and d == 0),
                                                     stop=(src == 1 and d == 2))
                            _copy(nc, sy[:, fo, pr, :aa, :], ps[:, :aa, :])
                    qsb = pool.tile([128, 2 * RD, W], BF, tag="qE")
                    for pr in range(2):
                        for pc in range(2):
                            ps = pp.tile([128, RD, 96], F32, tag="ps")
                            for src in range(2):
                                for d in range(3):
                                    ti = 6 * src + (2 * d + 1 - pc)
                                    nc.tensor.matmul(ps[:, :aa, :], t_de[:, ti, :],
                                                     sy[:, src, pr, :aa, d:d + 96],
                                                     start=(src == 0 and d == 0),
                                                     stop=(src == 1 and d == 2))
                            dst = qsb.rearrange("p (r two) w -> p two r w", two=2)[:, pr, :aa]
                            dst2 = dst.rearrange("p r (j two) -> p two r j", two=2)[:, pc]
                            _copy(nc, dst2, ps[:, :aa, :])
                    sqq = pool.tile([128, 2 * RD, W], BF, tag="sqq")
                    nc.vector.scalar_tensor_tensor(sqq[:, :2 * aa, :], qsb[:, :2 * aa, :], 1.0,
                                                   qsb[:, :2 * aa, :], mybir.AluOpType.mult,
                                                   mybir.AluOpType.mult,
                                                   accum_out=qnp[:, a0 // RD:a0 // RD + 1])
                    nc.sync.dma_start(out=qd[:, 2 * a0:2 * a0 + 2 * aa, :], in_=qsb[:, :2 * aa, :])

            _mark(nc, "gram")
            # ======== gram + attention block
            with tc.tile_pool(name="gw", bufs=1) as gw, \
                 tc.tile_pool(name="gps", bufs=1, space="PSUM") as gp:
                qdT = gw.tile([128, 288, 128], BF)
                g_ps = gp.tile([128, 128], F32, tag="g")
                qd_flat = qd.rearrange("p h w -> p (h w)")
                for c in range(8):
                    nc.sync.dma_start_transpose(out=qdT[:, 36 * c:36 * c + 36, :],
                                                in_=qd_flat[:, 4608 * c:4608 * c + 4608])
                    for i in range(36 * c, 36 * c + 36):
                        nc.tensor.matmul(g_ps[:, :], qdT[:, i, :], kdT[:, i, :],
                                         start=(i == 0), stop=(i == 287))

                _mark(nc, "attn")
                qn = gw.tile([128, 1], F32)
                nc.vector.tensor_reduce(qn[:, :], qnp[:, :], axis=mybir.AxisListType.X,
                                        op=mybir.AluOpType.add)
                nc.scalar.sqrt(qn[:, :], qn[:, :])
                nc.vector.tensor_scalar_max(qn[:, :], qn[:, :], 1e-12)
                rq = gw.tile([128, 1], F32)
                nc.vector.reciprocal(rq[:, :], qn[:, :])
                nc.vector.tensor_mul(rq[:, :], rq[:, :], t_temp[:, :])

                gsb = gw.tile([128, 128], BF)
                nc.vector.tensor_scalar_mul(gsb[:, :], g_ps[:, :], rq[:, :])
                pt = gp.tile([128, 128], BF, tag="pt", bufs=2)
                nc.tensor.transpose(pt[:, :], gsb[:, :], t_id[:, :])
                gtb = gw.tile([128, 128], BF)
                nc.scalar.activation(gtb[:, :], pt[:, :], mybir.ActivationFunctionType.Copy,
                                     scale=rk[:, :])
                pt2 = gp.tile([128, 128], BF, tag="pt", bufs=2)
                nc.tensor.transpose(pt2[:, :], gtb[:, :], t_id[:, :])

                eb = gw.tile([128, 32], F32)
                for h in range(4):
                    nc.scalar.activation(eb[32 * h:32 * h + 32, :],
                                         pt2[32 * h:32 * h + 32, 32 * h:32 * h + 32],
                                         mybir.ActivationFunctionType.Exp)
                ssum = gw.tile([128, 1], F32)
                nc.vector.tensor_reduce(ssum[:, :], eb[:, :], axis=mybir.AxisListType.X,
                                        op=mybir.AluOpType.add)
                rs = gw.tile([128, 1], F32)
                nc.vector.reciprocal(rs[:, :], ssum[:, :])
                nc.vector.tensor_scalar_mul(eb[:, :], eb[:, :], rs[:, :])
                bd = gw.tile([128, 128], BF)
                nc.vector.memset(bd[:, :], 0)
                for h in range(4):
                    nc.vector.tensor_copy(bd[32 * h:32 * h + 32, 32 * h:32 * h + 32],
                                          eb[32 * h:32 * h + 32, :])

                mps = gp.tile([128, 256], F32, tag="mps")
                nc.tensor.matmul(mps[:, :], bd[:, :], t_proj[:, :], start=True, stop=True)
                nc.scalar.copy(mt_[:, :], mps[:, :])

            _mark(nc, "y")
            # ======== y = M @ v (2KB-per-partition DMA granularity)
            with tc.tile_pool(name="yw", bufs=2) as yp, \
                 tc.tile_pool(name="yps", bufs=8, space="PSUM") as yq:
                for i in range(18):
                    vt = yp.tile([128, 2048], BF, tag="vt", bufs=10)
                    nc.sync.dma_start(out=vt[:, :], in_=vd[:, 2048 * i:2048 * i + 2048])
                    yst = [yp.tile([128, 2048], BF, tag="yst0", name="yst0", bufs=3),
                           yp.tile([128, 2048], BF, tag="yst1", name="yst1", bufs=3)]
                    for j in range(4):
                        for mtile in range(2):
                            ps = yq.tile([128, 512], F32, tag="ps")
                            nc.tensor.matmul(ps[:, :], mt_[:, 128 * mtile:128 * mtile + 128],
                                             vt[:, 512 * j:512 * j + 512], start=True, stop=True)
                            _copy(nc, yst[mtile][:, 512 * j:512 * j + 512], ps[:, :])
                    for mtile in range(2):
                        nc.sync.dma_start(out=y[mtile, :, 2048 * i:2048 * i + 2048],
                                          in_=yst[mtile][:, :])
    return nc


def _prep_core(x, qkv_w, qkv_conv_w, conv5_w, conv7_w, conv9_w, proj_w, temperature, b, g):
    bf = ml_dtypes.bfloat16
    xb = np.asarray(x[b], np.float32)
    sl = slice(128 * g, 128 * g + 128)
    qkv_loc = np.concatenate([qkv_w[sl], qkv_w[256 + 128 * g:256 + 128 * g + 128]], 0)  # (256 out, 256 in)
    conv_loc = np.concatenate([qkv_conv_w[sl, 0], qkv_conv_w[256 + 128 * g:256 + 128 * g + 128, 0]], 0)  # (256,3,3)
    w1 = qkv_loc.T.reshape(2, 128, 256).copy()  # [in-half, in(128), out(256)]
    dwk = np.zeros((9, 128, 128), np.float32)
    dwv = np.zeros((9, 128, 128), np.float32)
    for t in range(9):
        u, v = divmod(t, 3)
        dwk[t] = np.diag(conv_loc[:128, u, v])
        dwv[t] = np.diag(conv_loc[128:, u, v])
    taps_ab = np.zeros((12, 128, 128), np.float32)
    taps_de = np.zeros((12, 128, 128), np.float32)
    eye = np.eye(128, dtype=np.float32)
    for t in range(6):
        taps_ab[t] = eye * H0A[t]
        taps_ab[6 + t] = eye * H1A[t]
        taps_de[t] = eye * G0S[t]
        taps_de[6 + t] = eye * G1S[t]
    dwq = np.zeros((4, 9, 128, 128), np.float32)
    wq = {0: conv5_w, 1: conv5_w, 2: conv7_w, 3: conv9_w}
    for sb in range(4):
        wloc = wq[sb][sl, 0]
        for t in range(9):
            dwq[sb, t] = np.diag(wloc[:, t // 3, t % 3])
    projlt = proj_w[:, sl].T.copy()  # (128, 256)
    tempv = np.repeat(np.asarray(temperature).reshape(8)[4 * g:4 * g + 4], 32).astype(np.float32)[:, None]
    return {
        "xk0": xb[:128].astype(bf), "xk1": xb[128:].astype(bf), "xq": xb[sl].astype(bf),
        "w1x1": w1.astype(bf), "dwk": dwk.astype(bf), "dwv": dwv.astype(bf),
        "taps_ab": taps_ab.astype(bf), "taps_de": taps_de.astype(bf),
        "dwq": dwq.astype(bf), "projlt": projlt.astype(bf), "tempv": tempv,
        "identb": np.eye(128, dtype=np.float32).astype(bf),
    }


def kernel(x, qkv_w, qkv_conv_w, conv5_w, conv7_w, conv9_w, proj_w, temperature, num_heads):
    x = np.asarray(x, np.float32)
    args = [np.asarray(a, np.float32) for a in
            (qkv_w, qkv_conv_w, conv5_w, conv7_w, conv9_w, proj_w)]
    temperature = np.asarray(temperature, np.float32)
    nc = build_core_kernel()
    in_maps = [_prep_core(x, *args, temperature, core // 2, core % 2) for core in range(8)]
    res = run_bass_kernel_spmd(nc, in_maps, core_ids=list(range(8)))
    out = np.zeros((4, 256, H, W), np.float32)
    for b in range(4):
        acc = res.results[2 * b]["y"].astype(np.float32) + res.results[2 * b + 1]["y"].astype(np.float32)
        out[b] = acc.reshape(256, H, W)
    return out



# revision 2
# speedup vs baseline: 1.1005x; 1.1005x over previous
"""MDTA Trainium2 kernel v2: 8 cores = 4 samples x 2 head-groups.

fp8 DoubleRow matmuls (0.5 cyc/row) with overlapping-AP tap pairs for the
depthwise/wavelet convolutions; residual-fp8 compensation on the v-path
(x-residual + W1-residual in the 1x1, z-residual + dw-weight-residual in the
v depthwise) so quantization error stays ~bf16 on the output path; q/k path
runs plain fp8 (per-element noise washes out in the 36864-dim gram inner
products). Intermediates SBUF-resident except a bf16 v spill. PSUM evac on
DVE/Act/Pool; PE transposes for qT/kT (no DMA transpose).
"""
import numpy as np
import ml_dtypes
import json as _json
import concourse.bass as bass

# Patch Bass.to_json_bytes: split multi-sem-waits onto same-engine NoOps
# (this walrus build rejects instructions with >1 sync wait).
_orig_tjb = bass.Bass.to_json_bytes
_wctr = [0]


def _split_waits(block):
    out = []
    for ins in block.get("instructions", []):
        si = ins.get("sync_info")
        waits = (si or {}).get("on_wait") or []
        if len(waits) > 1:
            si["on_wait"] = waits[-1:]
            for w in waits[:-1]:
                _wctr[0] += 1
                out.append({"debug": ins.get("debug", 0), "engine": ins["engine"],
                            "ins": [], "outs": [], "name": f"wsplit-{_wctr[0]}",
                            "opcode": "NoOp",
                            "sync_info": {"on_update": [], "on_wait": [w]}})
        out.append(ins)
    block["instructions"] = out
    for sub in block.get("blocks", []):
        _split_waits(sub)


def _patched_tjb(self):
    d = _json.loads(_orig_tjb(self))
    for fn in d.get("functions", []):
        for b in fn.get("blocks", []):
            _split_waits(b)
    return _json.dumps(d).encode()


if not getattr(bass.Bass, "_waitfix_done", False):
    bass.Bass.to_json_bytes = _patched_tjb
    bass.Bass._waitfix_done = True

import concourse.mybir as mybir
from concourse.tile import TileContext
from concourse.bass_utils import run_bass_kernel_spmd

BF = mybir.dt.bfloat16
F32 = mybir.dt.float32
F8 = mybir.dt.float8e4
H, W, C = 192, 192, 256
HW = H * W
S = 98
DR = mybir.MatmulPerfMode.DoubleRow
MUL = mybir.AluOpType.mult
ADD = mybir.AluOpType.add
SUB = mybir.AluOpType.subtract

DEC_LO = np.array([0.035226291882100656, -0.085441273882241486, -0.13501102001039084,
                   0.45987750211933132, 0.80689150931333875, 0.33267055295095688], dtype=np.float64)
DEC_HI = np.array([-0.33267055295095688, 0.80689150931333875, -0.45987750211933132,
                   -0.13501102001039084, 0.085441273882241486, 0.035226291882100656], dtype=np.float64)
H0A = DEC_LO[::-1].copy()
H1A = DEC_HI[::-1].copy()
G0S = DEC_LO.copy()  # REC_LO reversed = DEC_LO
G1S = np.array([0.035226291882100656, 0.085441273882241486, -0.13501102001039084,
                -0.45987750211933132, 0.80689150931333875, -0.33267055295095688], dtype=np.float64)[::-1].copy()

SW1, SDW, SAB, SC = 16., 8., 4., 8.
PAIRS9 = [((0, 0), (0, 1)), ((1, 0), (1, 1)), ((2, 0), (2, 1)), ((0, 2), (1, 2)), ((2, 2), None)]
ZROW = 194          # padded z row length
ZLEN = 1 + 14 * ZROW + 5  # guard + 14 rows + tail guard


def pair2(ap, delta):
    """Insert a (delta, 2) DoubleRow pair dim right after the partition dim."""
    ap2 = ap.unsqueeze(1)
    v = ap2.ap
    v[1] = (delta, 2)
    ap2.ap = v
    return ap2



def _copy_on(nc, eng, out, in_):
    if eng is nc.scalar:
        nc.scalar.copy(out, in_)
    else:
        eng.tensor_copy(out, in_)

def build_core_kernel():
    nc = bass.Bass("TRN2")
    x8 = nc.dram_tensor("x8", [128, 2, H, W], F8, kind="ExternalInput")
    xr8 = nc.dram_tensor("xr8", [128, 2, H, W], F8, kind="ExternalInput")
    w1k = nc.dram_tensor("w1k", [128, 2, 128], F8, kind="ExternalInput")
    w1v = nc.dram_tensor("w1v", [128, 2, 128], F8, kind="ExternalInput")
    w1vr = nc.dram_tensor("w1vr", [128, 2, 128], F8, kind="ExternalInput")
    dwk8p = nc.dram_tensor("dwk8p", [5, 128, 2, 128], F8, kind="ExternalInput")
    dwv8p = nc.dram_tensor("dwv8p", [5, 128, 2, 128], F8, kind="ExternalInput")
    dwvc = nc.dram_tensor("dwvc", [9, 128, 2, 128], F8, kind="ExternalInput")
    tapsAB = nc.dram_tensor("tapsAB", [2, 3, 128, 2, 128], F8, kind="ExternalInput")
    tapsC = nc.dram_tensor("tapsC", [4, 5, 128, 2, 128], F8, kind="ExternalInput")
    tapsD = nc.dram_tensor("tapsD", [2, 3, 128, 2, 128], F8, kind="ExternalInput")
    tapsE = nc.dram_tensor("tapsE", [2, 3, 128, 2, 128], F8, kind="ExternalInput")
    projbf = nc.dram_tensor("projbf", [128, 256], BF, kind="ExternalInput")
    tempv = nc.dram_tensor("tempv", [128, 1], F32, kind="ExternalInput")
    identb = nc.dram_tensor("identb", [128, 128], BF, kind="ExternalInput")
    y = nc.dram_tensor("y", [2, 128, HW], BF, kind="ExternalOutput")
    vspill = nc.dram_tensor("vspill", [128, HW], BF)

    with TileContext(nc) as tc:
        with tc.tile_pool(name="const", bufs=1) as cpool:
            t_w1k = cpool.tile([128, 2, 128], F8)
            nc.sync.dma_start(out=t_w1k[:, :, :], in_=w1k[:, :, :])
            t_w1v = cpool.tile([128, 2, 128], F8)
            nc.sync.dma_start(out=t_w1v[:, :, :], in_=w1v[:, :, :])
            t_w1vr = cpool.tile([128, 2, 128], F8)
            nc.sync.dma_start(out=t_w1vr[:, :, :], in_=w1vr[:, :, :])
            t_dwk = cpool.tile([128, 5, 2, 128], F8)
            nc.sync.dma_start(out=t_dwk[:, :, :, :], in_=dwk8p.rearrange("t p i c -> p t i c"))
            t_dwv = cpool.tile([128, 5, 2, 128], F8)
            nc.sync.dma_start(out=t_dwv[:, :, :, :], in_=dwv8p.rearrange("t p i c -> p t i c"))
            t_dwvc = cpool.tile([128, 9, 2, 128], F8)
            nc.sync.dma_start(out=t_dwvc[:, :, :, :], in_=dwvc.rearrange("t p i c -> p t i c"))
            t_ab = cpool.tile([128, 2, 3, 2, 128], F8)
            nc.scalar.dma_start(out=t_ab[:, :, :, :, :], in_=tapsAB.rearrange("f t p i c -> p f t i c"))
            t_c = cpool.tile([128, 4, 5, 2, 128], F8)
            nc.scalar.dma_start(out=t_c[:, :, :, :, :], in_=tapsC.rearrange("s t p i c -> p s t i c"))
            t_d = cpool.tile([128, 2, 3, 2, 128], F8)
            nc.scalar.dma_start(out=t_d[:, :, :, :, :], in_=tapsD.rearrange("f t p i c -> p f t i c"))
            t_e = cpool.tile([128, 2, 3, 2, 128], F8)
            nc.scalar.dma_start(out=t_e[:, :, :, :, :], in_=tapsE.rearrange("f t p i c -> p f t i c"))
            t_proj = cpool.tile([128, 256], BF)
            nc.scalar.dma_start(out=t_proj[:, :], in_=projbf[:, :])
            t_temp = cpool.tile([128, 1], F32)
            nc.scalar.dma_start(out=t_temp[:, :], in_=tempv[:, :])
            t_id = cpool.tile([128, 128], BF)
            nc.scalar.dma_start(out=t_id[:, :], in_=identb[:, :])
            t_id8 = cpool.tile([128, 128], F8)
            nc.vector.tensor_copy(t_id8[:, :], t_id[:, :])

            knp = cpool.tile([128, 16], F32)
            qnp = cpool.tile([128, 20], F32)
            nc.vector.memset(knp[:, :], 0)
            nc.vector.memset(qnp[:, :], 0)
            kdT = cpool.tile([128, 288, 128], F8)
            mt_ = cpool.tile([128, 256], BF)

            # ================= P1: z (1x1) + dw3x3 + kT ==================
            RB1, NB1 = 12, 16
            with tc.tile_pool(name="p1", bufs=2) as p1, \
                 tc.tile_pool(name="p1ps", bufs=1, space="PSUM") as pp1:
                for b in range(NB1):
                    r0 = RB1 * b
                    zlo = r0 - 1                      # z rows zlo .. zlo+13
                    v0, v1 = max(0, zlo), min(H, zlo + 14)
                    xb = p1.tile([128, 2, 14, W], F8, tag="xb")
                    xrb = p1.tile([128, 2, 14, W], F8, tag="xrb")
                    if b == 0:
                        nc.vector.memset(xb[:, :, 0, :], 0)
                        nc.vector.memset(xrb[:, :, 0, :], 0)
                    if b == NB1 - 1:
                        nc.vector.memset(xb[:, :, 13, :], 0)
                        nc.vector.memset(xrb[:, :, 13, :], 0)
                    nc.sync.dma_start(out=xb[:, :, v0 - zlo:v1 - zlo, :], in_=x8[:, :, v0:v1, :])
                    nc.scalar.dma_start(out=xrb[:, :, v0 - zlo:v1 - zlo, :], in_=xr8[:, :, v0:v1, :])
                    xbf = xb.rearrange("p i r w -> p i (r w)")
                    xrf = xrb.rearrange("p i r w -> p i (r w)")

                    # z tiles: flat padded geometry (guard + 14*194 + guard)
                    z8k = p1.tile([128, ZLEN], F8, tag="z8k")
                    zv = p1.tile([128, 2, ZLEN], F8, tag="zv")
                    # zero guards + pad columns (cols 0 and 193 of each row)
                    nc.vector.memset(z8k[:, 0:1], 0)
                    nc.vector.memset(z8k[:, 1 + 14 * ZROW:], 0)
                    zpad = z8k[:, 1:1 + 14 * ZROW].rearrange("p (r w) -> p r w", r=14)
                    nc.vector.memset(zpad[:, :, 0:1], 0)
                    nc.vector.memset(zpad[:, :, 193:194], 0)
                    nc.vector.memset(zv[:, :, 0:1], 0)
                    nc.vector.memset(zv[:, :, 1 + 14 * ZROW:], 0)
                    zvpad = zv[:, :, 1:1 + 14 * ZROW].rearrange("p i (r w) -> p i r w", r=14)
                    nc.vector.memset(zvpad[:, :, :, 0:1], 0)
                    nc.vector.memset(zvpad[:, :, :, 193:194], 0)

                    # ---- z matmuls: 2-row groups, psum [128,3,512] (3 banks)
                    for half in range(2):
                        for ct in range(3):           # tiles of (6,6,2) rows
                            nrow = 2 if ct == 2 else 6
                            ps = pp1.tile([128, 3, 512], F32, tag="zdps", bufs=2, name="zps")
                            for gidx in range(nrow // 2):
                                r = 6 * ct + 2 * gidx
                                rhs = xbf[:, :, 192 * r:192 * (r + 2)]
                                o = ps[:, gidx, 0:384]
                                if half == 0:
                                    nc.tensor.matmul(o, t_w1k[:, :, :], rhs, start=True,
                                                     stop=True, perf_mode=DR, skip_group_check=True)
                                else:
                                    rhs_r = xrf[:, :, 192 * r:192 * (r + 2)]
                                    nc.tensor.matmul(o, t_w1v[:, :, :], rhs, start=True,
                                                     stop=False, perf_mode=DR, skip_group_check=True)
                                    nc.tensor.matmul(o, t_w1v[:, :, :], rhs_r, start=False,
                                                     stop=False, perf_mode=DR, skip_group_check=True)
                                    nc.tensor.matmul(o, t_w1vr[:, :, :], rhs, start=False,
                                                     stop=True, perf_mode=DR, skip_group_check=True)
                            # evac into padded flat z (rows 6ct..6ct+nrow)
                            ng = nrow // 2
                            src = ps[:, 0:ng, 0:384].rearrange("p g (r w) -> p g r w", r=2)
                            dz = zpad if half == 0 else zvpad[:, 0]
                            dst = dz[:, 6 * ct:6 * ct + nrow, 1:193].rearrange(
                                "p (g r) w -> p g r w", g=ng)
                            if half == 0:
                                _copy_on(nc, nc.scalar, dst, src)
                            else:
                                nc.vector.tensor_copy(dst, src)
                            if half == 1:
                                dr_ = zvpad[:, 1, 6 * ct:6 * ct + nrow, 1:193].rearrange(
                                    "p (g r) w -> p g r w", g=ng)
                                nc.vector.scalar_tensor_tensor(dr_, src, 1.0, dst, MUL, SUB)

                    # ---- dw3x3: flat-padded 2-row chunks (388 out elems)
                    kb = p1.tile([128, RB1, W], BF, tag="kb")
                    vb = p1.tile([128, RB1, W], BF, tag="vb")
                    for half in range(2):
                        for ct in range(2):           # 2 tiles x 3 groups x 2 rows
                            ps = pp1.tile([128, 3, 512], F32, tag="zdps", bufs=2, name="dps")
                            for gidx in range(3):
                                j = 6 * ct + 2 * gidx  # band-rel out rows j, j+1
                                base = 1 + (j + 1) * ZROW  # out flat start in z geometry
                                o = ps[:, gidx, 0:388]
                                if half == 0:
                                    for pi, (d1, d2) in enumerate(PAIRS9):
                                        u, vv = d1
                                        off = base + (u - 1) * ZROW + (vv - 1)
                                        win = z8k[:, off:off + 388]
                                        dd = 0 if d2 is None else (d2[0] - u) * ZROW + (d2[1] - vv)
                                        nc.tensor.matmul(o, t_dwk[:, pi, :, :], pair2(win, dd),
                                                         start=(pi == 0), stop=(pi == 4),
                                                         perf_mode=DR, skip_group_check=True)
                                else:
                                    for pi, (d1, d2) in enumerate(PAIRS9):
                                        u, vv = d1
                                        off = base + (u - 1) * ZROW + (vv - 1)
                                        win = zv[:, 0, off:off + 388]
                                        dd = 0 if d2 is None else (d2[0] - u) * ZROW + (d2[1] - vv)
                                        nc.tensor.matmul(o, t_dwv[:, pi, :, :], pair2(win, dd),
                                                         start=(pi == 0), stop=False,
                                                         perf_mode=DR, skip_group_check=True)
                                    for t9 in range(9):
                                        u, vv = divmod(t9, 3)
                                        off = base + (u - 1) * ZROW + (vv - 1)
                                        win = zv[:, 0, off:off + 388]
                                        nc.tensor.matmul(o, t_dwvc[:, t9, :, :], pair2(win, ZLEN),
                                                         start=False, stop=(t9 == 8),
                                                         perf_mode=DR, skip_group_check=True)
                            # evac: strip pad cols (valid at flat offsets 1..192, 195..386)
                            src = ps[:, :, 0:388].rearrange("p g (r w) -> p g r w", r=2)[:, :, :, 1:193]
                            dst = (kb if half == 0 else vb)[:, 6 * ct:6 * ct + 6, :].rearrange(
                                "p (g r) w -> p g r w", g=3)
                            nc.scalar.copy(dst, src)
                    # k-norm partial (gpsimd STT with scratch)
                    ksq = p1.tile([128, RB1, W], BF, tag="ksq", name="ksq")
                    nc.vector.scalar_tensor_tensor(ksq[:, :, :], kb[:, :, :], 1.0, kb[:, :, :],
                                                   MUL, MUL, accum_out=knp[:, b:b + 1])
                    nc.scalar.dma_start(out=vspill[:, r0 * W:(r0 + RB1) * W],
                                        in_=vb.rearrange("p r w -> p (r w)"))
                    # kT: 18 transpose chunks per band -> kdT (batches of 6)
                    kfl = kb.rearrange("p r w -> p (r w)")
                    for bt in range(3):
                        tps = pp1.tile([128, 6, 128], BF, tag="tps", bufs=2, name="tps")
                        for j in range(6):
                            ci = 6 * bt + j
                            nc.tensor.transpose(tps[:, j, :], kfl[:, 128 * ci:128 * ci + 128], t_id[:, :])
                        nc.vector.tensor_copy(kdT[:, 18 * b + 6 * bt:18 * b + 6 * bt + 6, :],
                                              tps[:, :, :])

            kn = cpool.tile([128, 1], F32)
            rk = cpool.tile([128, 1], F32)
            nc.vector.tensor_reduce(kn[:, :], knp[:, :], axis=mybir.AxisListType.X, op=ADD)
            nc.scalar.sqrt(kn[:, :], kn[:, :])
            nc.vector.tensor_scalar_max(kn[:, :], kn[:, :], 1e-12)
            nc.vector.reciprocal(rk[:, :], kn[:, :])

            # ================= P2a: stage A + B ==================
            with tc.tile_pool(name="p2ab", bufs=1) as pab, \
                 tc.tile_pool(name="p2a", bufs=2) as p2a, \
                 tc.tile_pool(name="p2aps", bufs=1, space="PSUM") as pp2a:
                aout = pab.tile([128, 2, 200, S], F8)   # rows offset +4
                nc.vector.memset(aout[:, :, 0:4, :], 0)
                nc.vector.memset(aout[:, :, 196:200, :], 0)
                # bout: [4, 101, 100] fp8 (row 0,99 + col 0,99 pads, +1 guard row)
                bout = pab.tile([128, 4, 101, 100], F8)
                nc.vector.memset(bout[:, :, 0, :], 0)
                nc.vector.memset(bout[:, :, 99:101, :], 0)
                nc.vector.memset(bout[:, :, :, 0:1], 0)
                nc.vector.memset(bout[:, :, :, 99:100], 0)
                # ---- A: per-row DR matmuls, 10-row bands
                r0 = 0
                while r0 < H:
                    rr = min(10, H - r0)
                    xqt = p2a.tile([128, 10, 202], F8, tag="xqt")
                    nc.vector.memset(xqt[:, :rr, 0:4], 0)
                    nc.vector.memset(xqt[:, :rr, 196:202], 0)
                    nc.sync.dma_start(out=xqt[:, :rr, 4:196], in_=x8[:, 0, r0:r0 + rr, :])
                    for ch in range(0, rr, 5):
                        hr = min(5, rr - ch)
                        ps = pp2a.tile([128, 2, 512], F32, tag="aps", bufs=2, name="aps")
                        for f in range(2):
                            for r in range(hr):
                                for pi in range(3):
                                    t = 2 * pi
                                    base = xqt[:, ch + r, t:t + 196]
                                    win = base.rearrange("p (j two) -> p two j", two=2)[:, 0]
                                    nc.tensor.matmul(ps[:, f, 98 * r:98 * r + 98],
                                                     t_ab[:, f, pi, :, :], pair2(win, 1),
                                                     start=(pi == 0), stop=(pi == 2),
                                                     perf_mode=DR, skip_group_check=True)
                        src = ps[:, :, 0:490].rearrange("p f (r j) -> p f r j", r=5)[:, :, :hr, :]
                        eng = nc.vector if (r0 // 10) % 2 == 0 else nc.scalar
                        _copy_on(nc, eng, aout[:, :, 4 + r0 + ch:4 + r0 + ch + hr, :], src)
                    r0 += rr

                # ---- B: per-m-row DR matmuls (pair delta = 98 rows)
                for m0 in range(0, S, 5):
                    mm = min(5, S - m0)
                    for srcf in range(2):
                        ps = pp2a.tile([128, 2, 512], F32, tag="bps", bufs=2, name="bps")
                        for f in range(2):
                            for r in range(mm):
                                m = m0 + r
                                for pi in range(3):
                                    t = 2 * pi
                                    win = aout[:, srcf, 2 * m + t, :]
                                    nc.tensor.matmul(ps[:, f, 98 * r:98 * r + 98],
                                                     t_ab[:, f, pi, :, :], pair2(win, S),
                                                     start=(pi == 0), stop=(pi == 2),
                                                     perf_mode=DR, skip_group_check=True)
                        src = ps[:, :, 0:490].rearrange("p f (r j) -> p f r j", r=5)[:, :, :mm, :]
                        eng = nc.scalar if srcf == 0 else nc.vector
                        # dest cols 1..98 (valid), rows 1+m0..
                        _copy_on(nc, eng, bout[:, 2 * srcf:2 * srcf + 2, 1 + m0:1 + m0 + mm, 1:99],
                                 src)

            # ================= P2b: stage C ==================
            # cout: [4, 98*98 + 4] fp8 (flat, +guard)
            cout = cpool.tile([128, 4, S * S + 4], F8)
            nc.vector.memset(cout[:, :, S * S:], 0)
            boutf = bout.rearrange("p s r w -> p s (r w)")
            with tc.tile_pool(name="p2bps", bufs=1, space="PSUM") as pp2b:
                for m0 in range(0, S, 5):
                    mm = min(5, S - m0)
                    L = mm * 100
                    ps = pp2b.tile([128, 4, 512], F32, tag="cps", bufs=2, name="cps")
                    for sb in range(4):
                        # out flat (100-geom) rows m0..m0+mm-1; bout row = m+u (pad +1 offset: row m+u at tensor row m+u)
                        obase = 0
                        for pi, (d1, d2) in enumerate(PAIRS9):
                            u, vv = d1
                            off = (m0 + u) * 100 + vv
                            win = boutf[:, sb, off:off + L]
                            dd = 0 if d2 is None else (d2[0] - u) * 100 + (d2[1] - vv)
                            nc.tensor.matmul(ps[:, sb, 0:L], t_c[:, sb, pi, :, :],
                                             pair2(win, dd), start=(pi == 0), stop=(pi == 4),
                                             perf_mode=DR, skip_group_check=True)
                    # evac: valid cols 1..98 of 100-geom -> cout flat 98-geom
                    src = ps[:, :, 0:L].rearrange("p s (r w) -> p s r w", r=mm)[:, :, :, 0:98]
                    dst = cout[:, :, m0 * S:(m0 + mm) * S].rearrange("p s (r w) -> p s r w", r=mm)
                    eng = nc.scalar if (m0 // 5) % 2 == 0 else nc.vector
                    _copy_on(nc, eng, dst, src)

            # ================= P2c: D + E + qT + gram ==================
            with tc.tile_pool(name="p2c", bufs=2) as p2c, \
                 tc.tile_pool(name="p2cps", bufs=1, space="PSUM") as pp2c:
                g_ps = pp2c.tile([128, 128], F32, tag="gram", bufs=1, name="gram")
                ci_total = 0
                a0 = 0
                blk = 0
                SYL = 5 * S + 4
                while a0 < 96:
                    aa = min(5, 96 - a0)
                    L = aa * S
                    sy = p2c.tile([128, 2, 2, SYL], F8, tag="sy")
                    if a0 < 10:
                        nc.vector.memset(sy[:, :, :, :], 0)
                    # ---- D: psum [2pr, aa*98] per fo
                    for fo in range(2):
                        ps = pp2c.tile([128, 2, 512], F32, tag="dps2", bufs=2, name="dps2")
                        for pr in range(2):
                            wins = [cout[:, 2 * fo, (a0 + 0) * S:(a0 + 0) * S + L],
                                    cout[:, 2 * fo, (a0 + 2) * S:(a0 + 2) * S + L],
                                    cout[:, 2 * fo + 1, (a0 + 1) * S:(a0 + 1) * S + L]]
                            deltas = [S, (S * S + 4) - 2 * S, S]
                            for pi in range(3):
                                nc.tensor.matmul(ps[:, pr, 0:L], t_d[:, pr, pi, :, :],
                                                 pair2(wins[pi], deltas[pi]),
                                                 start=(pi == 0), stop=(pi == 2),
                                                 perf_mode=DR, skip_group_check=True)
                        eng = nc.vector if fo == 0 else nc.scalar
                        _copy_on(nc, eng, sy[:, fo, :, 0:L], ps[:, :, 0:L])
                    # ---- E: psum [2pc, aa*98(96 valid)] per pr
                    qb = p2c.tile([128, 10, W], BF, tag="qb")
                    for pr in range(2):
                        ps = pp2c.tile([128, 2, 512], F32, tag="eps", bufs=1, name="eps")
                        for pc in range(2):
                            wins = [sy[:, 0, pr, 0:L], sy[:, 0, pr, 2:2 + L],
                                    sy[:, 1, pr, 1:1 + L]]
                            deltas = [1, 2 * SYL - 2, 1]
                            for pi in range(3):
                                nc.tensor.matmul(ps[:, pc, 0:L], t_e[:, pc, pi, :, :],
                                                 pair2(wins[pi], deltas[pi]),
                                                 start=(pi == 0), stop=(pi == 2),
                                                 perf_mode=DR, skip_group_check=True)
                        # evac: psum (pc, a, j<96) -> qb[2a+pr, 2j+pc]
                        src = ps[:, :, 0:L].rearrange("p c (a j) -> p c a j", a=aa)[:, :, :, 0:96]
                        dst = qb[:, pr:2 * aa:2, :].rearrange("p r (j two) -> p two r j", two=2)
                        _copy_on(nc, nc.scalar, dst, src)
                    # q-norm partial
                    qsq = p2c.tile([128, 10, W], BF, tag="qsq", name="qsq")
                    nc.vector.scalar_tensor_tensor(qsq[:, :2 * aa, :], qb[:, :2 * aa, :], 1.0,
                                                   qb[:, :2 * aa, :], MUL, MUL,
                                                   accum_out=qnp[:, blk:blk + 1])
                    # qT + gram
                    qfl = qb.rearrange("p r w -> p (r w)")
                    nch = (2 * aa * W) // 128
                    for bt in range(0, nch, 5):
                        nb = min(5, nch - bt)
                        tps = pp2c.tile([128, 5, 128], BF, tag="qtps", bufs=1, name="qtps")
                        for j in range(nb):
                            ci = bt + j
                            nc.tensor.transpose(tps[:, j, :], qfl[:, 128 * ci:128 * ci + 128],
                                                t_id[:, :])
                        qtc = p2c.tile([128, 5, 128], F8, tag="qtc", bufs=3)
                        nc.vector.tensor_copy(qtc[:, :nb, :], tps[:, :nb, :])
                        for j in range(nb):
                            nc.tensor.matmul(g_ps[:, :], qtc[:, j, :], kdT[:, ci_total + j, :],
                                             start=(ci_total + j == 0), stop=(ci_total + j == 287))
                        ci_total += nb
                    a0 += aa
                    blk += 1

                # gram normalization on q side (needs g_ps before pool closes)
                qn = cpool.tile([128, 1], F32)
                nc.vector.tensor_reduce(qn[:, :], qnp[:, :], axis=mybir.AxisListType.X, op=ADD)
                nc.scalar.sqrt(qn[:, :], qn[:, :])
                nc.vector.tensor_scalar_max(qn[:, :], qn[:, :], 1e-12)
                rq = cpool.tile([128, 1], F32)
                nc.vector.reciprocal(rq[:, :], qn[:, :])
                nc.vector.tensor_mul(rq[:, :], rq[:, :], t_temp[:, :])
                gsb = cpool.tile([128, 128], BF)
                nc.vector.tensor_scalar_mul(gsb[:, :], g_ps[:, :], rq[:, :])

            # ================= attention ==================
            with tc.tile_pool(name="attnps", bufs=1, space="PSUM") as ppat:
                pt = ppat.tile([128, 128], BF, tag="pt", bufs=2)
                nc.tensor.transpose(pt[:, :], gsb[:, :], t_id[:, :])
                gtb = cpool.tile([128, 128], BF)
                nc.scalar.activation(gtb[:, :], pt[:, :], mybir.ActivationFunctionType.Copy,
                                     scale=rk[:, :])
                pt2 = ppat.tile([128, 128], BF, tag="pt", bufs=2)
                nc.tensor.transpose(pt2[:, :], gtb[:, :], t_id[:, :])
                eb = cpool.tile([128, 32], F32)
                for h in range(4):
                    nc.scalar.activation(eb[32 * h:32 * h + 32, :],
                                         pt2[32 * h:32 * h + 32, 32 * h:32 * h + 32],
                                         mybir.ActivationFunctionType.Exp)
                ssum = cpool.tile([128, 1], F32)
                nc.vector.tensor_reduce(ssum[:, :], eb[:, :], axis=mybir.AxisListType.X, op=ADD)
                rs = cpool.tile([128, 1], F32)
                nc.vector.reciprocal(rs[:, :], ssum[:, :])
                nc.vector.tensor_scalar_mul(eb[:, :], eb[:, :], rs[:, :])
                bd = cpool.tile([128, 128], BF)
                nc.vector.memset(bd[:, :], 0)
                for h in range(4):
                    nc.vector.tensor_copy(bd[32 * h:32 * h + 32, 32 * h:32 * h + 32],
                                          eb[32 * h:32 * h + 32, :])
                mps = ppat.tile([128, 256], F32, tag="mps", bufs=1)
                nc.tensor.matmul(mps[:, :], bd[:, :], t_proj[:, :], start=True, stop=True)
                nc.scalar.copy(mt_[:, :], mps[:, :])

            # ================= P3: y = M @ v ==================
            with tc.tile_pool(name="p3", bufs=2) as p3, \
                 tc.tile_pool(name="p3ps", bufs=1, space="PSUM") as pp3:
                for i in range(18):
                    vt = p3.tile([128, 2048], BF, tag="vt", bufs=3)
                    nc.scalar.dma_start(out=vt[:, :], in_=vspill[:, 2048 * i:2048 * i + 2048])
                    for mtile in range(2):
                        ps = pp3.tile([128, 4, 512], F32, tag="yps", bufs=2, name="yps")
                        for j in range(4):
                            nc.tensor.matmul(ps[:, j, :], mt_[:, 128 * mtile:128 * mtile + 128],
                                             vt[:, 512 * j:512 * j + 512], start=True, stop=True,
                                             skip_group_check=True)
                        yst = p3.tile([128, 2048], BF, tag=f"yst{mtile}", name="yst", bufs=3)
                        eng = [nc.vector, nc.scalar][mtile]
                        _copy_on(nc, eng, yst[:, :], ps.rearrange("p a b -> p (a b)"))
                        nc.sync.dma_start(out=y[mtile, :, 2048 * i:2048 * i + 2048], in_=yst[:, :])
    return nc


def _q8(a):
    return np.asarray(a, np.float32).astype(ml_dtypes.float8_e4m3)


def _diag_pairs(vals_pairs):
    out = np.zeros((len(vals_pairs), 128, 2, 128), np.float32)
    eye = np.eye(128, dtype=np.float32)
    for i, (v0, v1) in enumerate(vals_pairs):
        out[i, :, 0, :] = eye * v0
        out[i, :, 1, :] = eye * v1
    return _q8(out)


def _prep_core(x, qkv_w, qkv_conv_w, conv5_w, conv7_w, conv9_w, proj_w, temperature, b, g):
    f8 = ml_dtypes.float8_e4m3
    bf = ml_dtypes.bfloat16
    xb = np.asarray(x[b], np.float32)
    sl = slice(128 * g, 128 * g + 128)
    go = 128 * (1 - g)
    xh = np.stack([xb[sl], xb[go:go + 128]], 1)  # [128, 2, H, W]; slice0 = q channels
    x8 = xh.astype(f8)
    xr8 = (xh - x8.astype(np.float32)).astype(f8)

    krows = np.arange(128 * g, 128 * g + 128)
    vrows = 256 + krows
    w16 = np.asarray(qkv_w, np.float32) * SW1

    def mk_w1(rows):
        w = np.zeros((128, 2, 128), np.float32)
        w[:, 0, :] = w16[np.ix_(rows, np.arange(128 * g, 128 * g + 128))].T
        w[:, 1, :] = w16[np.ix_(rows, np.arange(go, go + 128))].T
        return w

    w1k_f = mk_w1(krows)
    w1v_f = mk_w1(vrows)
    w1k_ = w1k_f.astype(f8)
    w1v_ = w1v_f.astype(f8)
    w1vr_ = (w1v_f - w1v_.astype(np.float32)).astype(f8)

    convw = np.asarray(qkv_conv_w, np.float32)[:, 0] * SDW
    dk = convw[krows]
    dv_f = convw[vrows]
    dk8 = dk.astype(f8).astype(np.float32)
    dv8 = dv_f.astype(f8).astype(np.float32)
    dvr = dv_f - dv8

    def tap_pack(wmat):
        out = np.zeros((5, 128, 2, 128), np.float32)
        for i, (d1, d2) in enumerate(PAIRS9):
            out[i, :, 0, :] = np.diag(wmat[:, d1[0], d1[1]])
            if d2 is not None:
                out[i, :, 1, :] = np.diag(wmat[:, d2[0], d2[1]])
        return _q8(out)

    dwk8p_ = tap_pack(dk8)
    dwv8p_ = tap_pack(dv8)
    dwvc_ = np.zeros((9, 128, 2, 128), np.float32)
    for t in range(9):
        u, vv = divmod(t, 3)
        dwvc_[t, :, 0, :] = np.diag(dvr[:, u, vv])
        dwvc_[t, :, 1, :] = np.diag(dv8[:, u, vv])
    dwvc_ = _q8(dwvc_)

    tapsAB_ = np.stack([
        _diag_pairs([(H0A[0] * SAB, H0A[1] * SAB), (H0A[2] * SAB, H0A[3] * SAB),
                     (H0A[4] * SAB, H0A[5] * SAB)]),
        _diag_pairs([(H1A[0] * SAB, H1A[1] * SAB), (H1A[2] * SAB, H1A[3] * SAB),
                     (H1A[4] * SAB, H1A[5] * SAB)]),
    ], 0)

    wq = {0: conv5_w, 1: conv5_w, 2: conv7_w, 3: conv9_w}
    tapsC_ = np.zeros((4, 5, 128, 2, 128), f8)
    for sb in range(4):
        wloc = np.asarray(wq[sb], np.float32)[sl, 0] * SC
        tapsC_[sb] = tap_pack(wloc.astype(f8).astype(np.float32))

    tapsD_ = np.stack([
        _diag_pairs([(G0S[1 - pr], G0S[3 - pr]), (G0S[5 - pr], G1S[1 - pr]),
                     (G1S[3 - pr], G1S[5 - pr])]) for pr in range(2)], 0)
    tapsE_ = np.stack([
        _diag_pairs([(G0S[1 - pc], G0S[3 - pc]), (G0S[5 - pc], G1S[1 - pc]),
                     (G1S[3 - pc], G1S[5 - pc])]) for pc in range(2)], 0)

    projlt = np.asarray(proj_w, np.float32)[:, sl].T.copy()
    tempvv = np.repeat(np.asarray(temperature).reshape(8)[4 * g:4 * g + 4], 32).astype(np.float32)[:, None]
    return {
        "x8": x8, "xr8": xr8, "w1k": w1k_, "w1v": w1v_, "w1vr": w1vr_,
        "dwk8p": dwk8p_, "dwv8p": dwv8p_, "dwvc": dwvc_,
        "tapsAB": tapsAB_, "tapsC": np.asarray(tapsC_), "tapsD": tapsD_, "tapsE": tapsE_,
        "projbf": projlt.astype(bf), "tempv": tempvv,
        "identb": np.eye(128, dtype=np.float32).astype(bf),
    }


def kernel(x, qkv_w, qkv_conv_w, conv5_w, conv7_w, conv9_w, proj_w, temperature, num_heads):
    x = np.asarray(x, np.float32)
    args = [np.asarray(a, np.float32) for a in
            (qkv_w, qkv_conv_w, conv5_w, conv7_w, conv9_w, proj_w)]
    temperature = np.asarray(temperature, np.float32)
    nc = build_core_kernel()
    in_maps = [_prep_core(x, *args, temperature, core // 2, core % 2) for core in range(8)]
    res = run_bass_kernel_spmd(nc, in_maps, core_ids=list(range(8)))
    out = np.zeros((4, 256, H, W), np.float32)
    scale = 1.0 / (SW1 * SDW)
    for b in range(4):
        acc = res.results[2 * b]["y"].astype(np.float32) + res.results[2 * b + 1]["y"].astype(np.float32)
        out[b] = acc.reshape(256, H, W) * scale
    return out
